# revision 20
# baseline (speedup 1.0000x reference)
"""Trainium2 Bass kernel for a dense transformer block (B=2,S=2048,D=1024,H=16,DFF=4096).

Sharding across 8 NeuronCores (no AllGathers):
  core c: batch b=c//4, group rank r=c%4, replica groups [[0,1,2,3],[4,5,6,7]].
  - x replicated within the group; LN1 + transpose computed redundantly over
    the full sequence (cheap) so QKV needs no collective.
  - Attention: head-parallel (4 heads/core, full causal sequence), fused in a
    per-query-strip pipeline with LN1/QKV.
  - out_proj: each core computes the partial y contribution of its heads for
    the strip, then a small ReduceScatter(add) both sums the partials and
    scatters tokens -- 4 chunked RS ops overlap with attention of later strips.
  - residual + LN2 + FFN: token-parallel on the core's 512 owned (interleaved)
    tokens with full FFN weights (no collective).
Matmul operands are bf16 (weights converted host-side); accumulation and the
residual spine stay fp32.
"""
import sys

sys.path.insert(0, "/opt/trn_rl_repo")

import numpy as np
import ml_dtypes

import concourse.bass as bass
import concourse.mybir as mybir
import concourse.tile as tile
from concourse import bacc
from concourse.bass_utils import run_bass_kernel_spmd
from concourse.masks import make_identity

AF = mybir.ActivationFunctionType
ALU = mybir.AluOpType
F32 = mybir.dt.float32
F32R = mybir.dt.float32r
BF16 = mybir.dt.bfloat16

B, S, D, H = 2, 2048, 1024, 16
DH = D // H          # 64
DFF = 4 * D          # 4096
EPS = 1e-5
NC = 8               # cores
G = 4                # cores per group (per batch)
TS = S // G          # 512 tokens per query strip
HC = H // G          # 4 heads per core
CC = HC * DH         # 256 head-columns per core
P = 128
KD = D // P          # 8 k-tiles over D
KF = DFF // P        # 32 k-tiles over DFF
NT = TS // P         # 4 token tiles per strip
GROUPS = [[0, 1, 2, 3], [4, 5, 6, 7]]
MFB = 4              # mf tiles per W1 stream block

_CACHE = {}


def build():
    nc = bacc.Bacc(None)

    io = {}
    dp = nc.declare_dram_parameter
    io["x_d"] = dp("x", [S, D], F32, isOutput=False)
    io["xr_d"] = dp("xr", [TS, D], F32, isOutput=False)
    io["wq_d"] = dp("Wq", [D, CC], BF16, isOutput=False)
    io["wk_d"] = dp("Wk", [D, CC], BF16, isOutput=False)
    io["wv_d"] = dp("Wv", [D, CC], BF16, isOutput=False)
    io["bq_d"] = dp("bq", [CC], F32, isOutput=False)
    io["bk_d"] = dp("bk", [CC], F32, isOutput=False)
    io["bv_d"] = dp("bv", [CC], F32, isOutput=False)
    io["wo_d"] = dp("Wo", [CC, D], BF16, isOutput=False)
    io["w1_d"] = dp("W1", [D, DFF], BF16, isOutput=False)
    io["b1_d"] = dp("b1", [DFF], F32, isOutput=False)
    io["w2_d"] = dp("W2", [DFF, D], BF16, isOutput=False)
    io["b2_d"] = dp("b2", [D], BF16, isOutput=False)
    io["y_d"] = dp("y", [TS, D], F32, isOutput=True)

    for qc in range(G):
        io[f"rs_in{qc}"] = nc.dram_tensor(f"rs_in{qc}", [TS, D], BF16)
        io[f"rs_out{qc}"] = nc.dram_tensor(f"rs_out{qc}", [P, D], BF16)

    with tile.TileContext(nc) as tc:
        _body(nc, tc, io)
    nc.compile()
    return nc


def _body(nc, tc, t):
    with tc.tile_pool(name="const", bufs=1) as cst:
        # ---------------- constants ----------------
        ident = cst.tile([P, P], F32)
        make_identity(nc, ident[:])

        ones128 = cst.tile([1, P], BF16)        # K=1 lhsT (M=128 tokens)
        nc.gpsimd.memset(ones128[:], 1.0)
        onescol4 = cst.tile([P, HC, 1], BF16)
        nc.gpsimd.memset(onescol4[:], 1.0)
        epsc = cst.tile([P, 1], F32)
        nc.gpsimd.memset(epsc[:], EPS)

        # doubled causal masks (one per diagonal shift), mask||mask layout so a
        # single DVE op masks a two-head [128, 1024] pair tile.
        maskd = {}
        for sh in (0, -128, -256, -384):
            md = cst.tile([P, 2 * TS], BF16, tag=f"maskd{sh}", name=f"maskd{sh}")
            nc.gpsimd.memset(md[:], 1.0)
            for half in range(2):
                nc.gpsimd.affine_select(
                    out=md[:, half * TS:(half + 1) * TS],
                    in_=md[:, half * TS:(half + 1) * TS],
                    compare_op=ALU.is_ge, fill=0.0, base=sh,
                    pattern=[[1, TS]], channel_multiplier=-1,
                )
            maskd[sh] = md

        # bq/bk as per-partition [128, 2] (column-tile-major) for psum eviction
        bqp = cst.tile([P, 2], F32)
        bkp = cst.tile([P, 2], F32)
        nc.gpsimd.dma_start(bqp[:], t["bq_d"].rearrange("(m p) -> p m", p=P))
        nc.gpsimd.dma_start(bkp[:], t["bk_d"].rearrange("(m p) -> p m", p=P))
        # bv broadcast across partitions for the v eviction add
        bvrow = cst.tile([1, CC], F32)
        nc.gpsimd.dma_start(bvrow[:], t["bv_d"][None, :])
        bvb = cst.tile([P, CC], F32)
        nc.gpsimd.partition_broadcast(bvb[:], bvrow[:])
        # b1 as per-partition [128, KF] for the gelu bias operand
        b1p = cst.tile([P, KF], F32)
        nc.gpsimd.dma_start(b1p[:], t["b1_d"].rearrange("(k p) -> p k", p=P))
        b2row = cst.tile([1, D], BF16)
        nc.gpsimd.dma_start(b2row[:], t["b2_d"][None, :])
        # residual strip (x at owned tokens, bo folded in host-side)
        xr = [cst.tile([P, D], F32, tag=f"xr{i}", name=f"xr{i}")
              for i in range(NT)]
        for i in range(NT):
            nc.gpsimd.dma_start(xr[i][:], t["xr_d"][i * P:(i + 1) * P, :])

        # ---------------- helpers ----------------
        def layernorm_tile(xt, dst, sc):
            # var = E[x^2] - mu^2 (safe: |mu| << std for this data); stats
            # chain kept on DVE to minimize cross-engine semaphore hops:
            # act does the big Square+accum pass and the Rsqrt table lookup.
            mu = sc.tile([P, 1], F32, tag="mu", name="mu")
            nc.vector.tensor_reduce(out=mu[:], in_=xt[:], op=ALU.add,
                                    axis=mybir.AxisListType.X)
            sq = sc.tile([P, D], F32, tag="sq", name="sq")
            sumsq = sc.tile([P, 1], F32, tag="sumsq", name="sumsq")
            nc.scalar.activation(sq[:], xt[:], AF.Square, accum_out=sumsq[:])
            mus = sc.tile([P, 1], F32, tag="mus", name="mus")
            nc.vector.tensor_scalar(out=mus[:], in0=mu[:], scalar1=1.0 / D,
                                    scalar2=None, op0=ALU.mult)
            mu2 = sc.tile([P, 1], F32, tag="mu2", name="mu2")
            nc.vector.tensor_tensor(out=mu2[:], in0=mus[:], in1=mus[:],
                                    op=ALU.mult)
            vpe = sc.tile([P, 1], F32, tag="vpe", name="vpe")
            nc.vector.tensor_scalar(out=vpe[:], in0=sumsq[:],
                                    scalar1=1.0 / D, scalar2=mu2[:],
                                    op0=ALU.mult, op1=ALU.subtract)
            std = sc.tile([P, 1], F32, tag="std", name="std")
            nc.scalar.activation(std[:], vpe[:], AF.Sqrt, bias=epsc[:])
            inv = sc.tile([P, 1], F32, tag="inv", name="inv")
            nc.vector.reciprocal(inv[:], std[:])
            nc.vector.tensor_scalar(out=dst[:], in0=xt[:],
                                    scalar1=mus[:], scalar2=inv[:],
                                    op0=ALU.subtract, op1=ALU.mult)

        # ============ phase 1: fused LN1/QKV/attention/out_proj + RS ========
        # pool entry order is chosen so exits can be LIFO
        sc_cm = tc.tile_pool(name="lnS", bufs=2)
        sc = sc_cm.__enter__()
        rsb_cm = tc.tile_pool(name="rsSB", bufs=4)
        rsb = rsb_cm.__enter__()
        pfw1_cm = tc.tile_pool(name="pfW1", bufs=1)
        pfw1 = pfw1_cm.__enter__()
        pps_cm = tc.tile_pool(name="psA", bufs=2, space="PSUM")
        pps = pps_cm.__enter__()
        x2p_cm = tc.tile_pool(name="x2P", bufs=1)
        x2p = x2p_cm.__enter__()
        qkv_cm = tc.tile_pool(name="qkvP", bufs=1)
        qkv = qkv_cm.__enter__()
        wp_cm = tc.tile_pool(name="wqkv", bufs=1)
        wp = wp_cm.__enter__()
        xsp_cm = tc.tile_pool(name="xsP", bufs=5)
        xsp = xsp_cm.__enter__()
        hTp_cm = tc.tile_pool(name="hTP", bufs=1)
        hTp = hTp_cm.__enter__()

        hT = hTp.tile([P, KD, S], BF16, tag="hT", name="hT")
        qT = [qkv.tile([P, S], BF16, tag=f"qT{m}", name=f"qT{m}") for m in range(2)]
        kT = [qkv.tile([P, S], BF16, tag=f"kT{m}", name=f"kT{m}") for m in range(2)]
        vo = [qkv.tile([P, HC, DH + 1], BF16, tag=f"vo{tm}", name=f"vo{tm}")
              for tm in range(S // P)]
        att = [qkv.tile([P, S], BF16, tag=f"att{hp}", name=f"att{hp}")
               for hp in range(2)]

        # first strip's x tiles; later strips prefetched inside the loop
        xs = {}
        for mt in range(NT):
            xs[mt] = xsp.tile([P, D], F32, tag="xs", name=f"xs{mt}")
            nc.sync.dma_start(xs[mt][:], t["x_d"][mt * P:(mt + 1) * P, :])

        # qkv weights + wo (sync queue, behind first x tiles)
        wq = [wp.tile([P, CC], BF16, tag=f"wq{k}", name=f"wq{k}") for k in range(KD)]
        wk = [wp.tile([P, CC], BF16, tag=f"wk{k}", name=f"wk{k}") for k in range(KD)]
        wv = [wp.tile([P, CC], BF16, tag=f"wv{k}", name=f"wv{k}") for k in range(KD)]
        for k in range(KD):
            nc.sync.dma_start(wq[k][:], t["wq_d"][k * P:(k + 1) * P, :])
            nc.sync.dma_start(wk[k][:], t["wk_d"][k * P:(k + 1) * P, :])
            nc.sync.dma_start(wv[k][:], t["wv_d"][k * P:(k + 1) * P, :])
        wo = [wp.tile([P, D], BF16, tag=f"wo{hp}", name=f"wo{hp}") for hp in range(2)]
        for hp in range(2):
            nc.sync.dma_start(wo[hp][:], t["wo_d"][hp * P:(hp + 1) * P, :])

        # FFN W1 prefetch: first block on the scalar queue
        w1s0 = [pfw1.tile([P, MFB * P], BF16, tag=f"w1s{k}", name=f"w1s{k}")
                for k in range(KD)]
        for k in range(KD):
            nc.scalar.dma_start(w1s0[k][:], t["w1_d"][k * P:(k + 1) * P,
                                                      0:MFB * P])

        # ---- LN1 + transpose + QKV, interleaved per strip ----
        for qc in range(G):
            if qc + 1 < G:
                for mt in range(NT):
                    mt16 = (qc + 1) * NT + mt
                    xs[mt16] = xsp.tile([P, D], F32, tag="xs", name=f"xs{mt16}")
                    nc.sync.dma_start(xs[mt16][:],
                                      t["x_d"][mt16 * P:(mt16 + 1) * P, :])
            for mt in range(NT):
                mt16 = qc * NT + mt
                h = sc.tile([P, D], F32, tag="h", name=f"h{mt16}", bufs=4)
                layernorm_tile(xs[mt16], h, sc)
                for g2 in range(2):        # k groups 0-3 / 4-7
                    ps = pps.tile([P, TS], F32, tag="ps", name="tps")
                    for kk in range(4):
                        k = 4 * g2 + kk
                        nc.tensor.transpose(ps[:, kk * P:(kk + 1) * P],
                                            h[:, k * P:(k + 1) * P],
                                            ident[:])
                    nc.vector.tensor_copy(
                        hT[:, 4 * g2:4 * g2 + 4,
                           mt16 * P:(mt16 + 1) * P],
                        ps[:].rearrange("p (k c) -> p k c", k=4))
            for (w_sb, b_sb, out_sb) in ((wq, bqp, qT), (wk, bkp, kT)):
                for m in range(2):
                    ps = pps.tile([P, TS], F32, tag="ps", name="pps")
                    for k in range(KD):
                        nc.tensor.matmul(
                            ps[:], w_sb[k][:, m * P:(m + 1) * P],
                            hT[:, k, qc * TS:(qc + 1) * TS],
                            start=(k == 0), stop=(k == KD - 1))
                    nc.vector.tensor_scalar(
                        out=out_sb[m][:, qc * TS:(qc + 1) * TS],
                        in0=ps[:], scalar1=b_sb[:, m:m + 1],
                        scalar2=None, op0=ALU.add)
            for mt in range(NT):
                tm = qc * NT + mt
                ps = pps.tile([P, TS], F32, tag="ps", name="vps")
                for k in range(KD):
                    nc.tensor.matmul(
                        ps[:, 0:CC], hT[:, k, tm * P:(tm + 1) * P],
                        wv[k][:], start=(k == 0), stop=(k == KD - 1))
                nc.vector.tensor_tensor(
                    out=vo[tm][:, :, 0:DH],
                    in0=ps[:, 0:CC].rearrange("p (h e) -> p h e", h=HC),
                    in1=bvb[:].rearrange("p (h e) -> p h e", h=HC),
                    op=ALU.add)
                nc.gpsimd.tensor_copy(vo[tm][:, :, DH:DH + 1], onescol4[:])

        # hT/xs dead after QKV; free them and open attention-phase pools
        hTp_cm.__exit__(None, None, None)
        xsp_cm.__exit__(None, None, None)
        asb_cm = tc.tile_pool(name="attnSB", bufs=2)
        asb = asb_cm.__enter__()
        scp_cm = tc.tile_pool(name="scPS", bufs=2, space="PSUM")
        scp = scp_cm.__enter__()
        avp_cm = tc.tile_pool(name="avPS", bufs=1, space="PSUM")
        avp = avp_cm.__enter__()
        x2 = [x2p.tile([P, D], F32, tag=f"x2{i}", name=f"x2{i}")
              for i in range(NT)]
        h2s = []

        def resid_ln2(pq):
            ro = rsb.tile([P, D], BF16, tag="ro", name=f"ro{pq}", bufs=1)
            nc.sync.dma_start(ro[:], t[f"rs_out{pq}"][:])
            nc.vector.tensor_tensor(out=x2[pq][:], in0=ro[:], in1=xr[pq][:],
                                    op=ALU.add)
            h2 = sc.tile([P, D], F32, tag="h2", name=f"h2{pq}", bufs=4)
            layernorm_tile(x2[pq], h2, sc)
            h2s.append(h2)

        for qc in range(G):
            # ---- attention for this strip (all 4 heads) ----
            kt_max = 4 * qc + 3
            for hp in range(HC // 2):      # head pairs at PE rows 0/64
                avs = avp.tile([DH + 1, 2, TS], F32, tag="av", name=f"av{hp}")
                for kt in range(kt_max + 1):
                    # diag blocks: only columns >= v0 are ever read
                    w0 = P * max(0, kt - 4 * qc)
                    sc_ps = scp.tile([P, 2, TS], F32, tag="scp", name="scp")
                    for j in range(2):
                        h_i = 2 * hp + j
                        m = h_i // 2
                        o = (h_i % 2) * DH
                        nc.tensor.matmul(
                            sc_ps[:, j, w0:],
                            kT[m][o:o + DH, kt * P:(kt + 1) * P],
                            qT[m][o:o + DH, qc * TS + w0:(qc + 1) * TS],
                            start=True, stop=True)
                    e_r = asb.tile([P, 2, TS], BF16, tag="erp", name="erp")
                    if kt < 4 * qc:
                        v0 = 0      # valid columns start
                        nc.scalar.activation(
                            e_r[:].rearrange("p a b -> p (a b)"),
                            sc_ps[:].rearrange("p a b -> p (a b)"),
                            AF.Exp, scale=0.125)
                    else:
                        # diag block, shift s=-128*d: cols < 128*d
                        # are fully masked -- never compute/read them
                        d = kt - 4 * qc
                        v0 = P * d
                        nc.scalar.activation(
                            e_r[:, :, v0:], sc_ps[:, :, v0:],
                            AF.Exp, scale=0.125)
                        sh = 512 * qc - 128 * kt
                        mdv = maskd[sh][:].rearrange("p (a b) -> p a b", a=2)
                        nc.vector.tensor_tensor(
                            out=e_r[:, :, v0:], in0=e_r[:, :, v0:],
                            in1=mdv[:, :, v0:], op=ALU.mult)
                    for j in range(2):
                        h_i = 2 * hp + j
                        nc.tensor.matmul(avs[:, j, v0:],
                                         vo[kt][:, h_i, :],
                                         e_r[:, j, v0:],
                                         start=(kt == 0),
                                         stop=(kt == kt_max))
                # normalize + write into persistent att tiles (bf16)
                un = asb.tile([DH + 1, 2, TS], F32, tag="un", name="un",
                              bufs=1)
                nc.vector.tensor_copy(un[:], avs[:])
                rec = asb.tile([1, 2, TS], F32, tag="rec", name="rec", bufs=1)
                nc.vector.reciprocal(rec[:].rearrange("p a b -> p (a b)"),
                                     un[DH:DH + 1, :, :].rearrange(
                                         "p a b -> p (a b)"))
                rb = asb.tile([DH, 2, TS], F32, tag="rb", name="rb", bufs=1)
                nc.gpsimd.partition_broadcast(
                    rb[:].rearrange("p a b -> p (a b)"),
                    rec[:].rearrange("p a b -> p (a b)"))
                for j in range(2):
                    nc.vector.tensor_tensor(
                        out=att[hp][j * DH:(j + 1) * DH,
                                    qc * TS:(qc + 1) * TS],
                        in0=un[0:DH, j, :], in1=rb[:, j, :], op=ALU.mult)

            # ---- out_proj partial for this strip + ReduceScatter ----
            for mt in range(NT):
                ri = rsb.tile([P, D], BF16, tag="ri", name=f"ri{qc}_{mt}")
                for n in range(2):
                    ps = pps.tile([P, TS], F32, tag="ps", name="ops")
                    for hp in range(2):
                        nc.tensor.matmul(
                            ps[:],
                            att[hp][:, qc * TS + mt * P:qc * TS + (mt + 1) * P],
                            wo[hp][:, n * TS:(n + 1) * TS],
                            start=(hp == 0), stop=(hp == 1))
                    nc.scalar.mul(ri[:, n * TS:(n + 1) * TS], ps[:], 1.0)
                nc.sync.dma_start(t[f"rs_in{qc}"][mt * P:(mt + 1) * P, :], ri[:])
            nc.gpsimd.collective_compute(
                "ReduceScatter", ALU.add, ins=[t[f"rs_in{qc}"][:]],
                outs=[t[f"rs_out{qc}"][:]], replica_groups=GROUPS,
            )
            if qc >= 1:
                resid_ln2(qc - 1)
        resid_ln2(3)

        # attention-phase pools done (LIFO)
        avp_cm.__exit__(None, None, None)
        scp_cm.__exit__(None, None, None)
        asb_cm.__exit__(None, None, None)
        wp_cm.__exit__(None, None, None)
        qkv_cm.__exit__(None, None, None)

        # ============ phase 2: FFN on owned tokens ========
        gtp_cm = tc.tile_pool(name="gTP", bufs=1)
        gtp = gtp_cm.__enter__()
        h2tp_cm = tc.tile_pool(name="h2TP", bufs=1)
        h2tp = h2tp_cm.__enter__()
        h2T = [h2tp.tile([P, TS], BF16, tag=f"h2T{k}", name=f"h2T{k}")
               for k in range(KD)]
        for k in range(KD):
            ps = pps.tile([P, TS], F32, tag="ps", name="tps2")
            for q3 in range(3):
                nc.tensor.transpose(ps[:, q3 * P:(q3 + 1) * P],
                                    h2s[q3][:, k * P:(k + 1) * P],
                                    ident[:])
            nc.vector.tensor_copy(h2T[k][:, 0:3 * P], ps[:, 0:3 * P])
        for k in range(KD):
            ps = pps.tile([P, TS], F32, tag="ps", name="tps3")
            nc.tensor.transpose(ps[:, 0:P],
                                h2s[3][:, k * P:(k + 1) * P], ident[:])
            nc.vector.tensor_copy(h2T[k][:, 3 * P:4 * P], ps[:, 0:P])
        # ---- FFN ----
        gT = [gtp.tile([P, TS], BF16, tag=f"gT{mf}", name=f"gT{mf}")
              for mf in range(KF)]
        w1p_cm = tc.tile_pool(name="w1st", bufs=1)
        w1p = w1p_cm.__enter__()
        for (c0t, c1t) in ((0, 3 * P), (3 * P, TS)):
            for blk in range(KF // MFB):
                if c0t == 0 and blk < 1:
                    w1s = [w1s0[k][:, 0:MFB * P] for k in range(KD)]
                else:
                    w1t = [w1p.tile([P, MFB * P], BF16,
                                    tag=f"w1b{k}", name=f"w1b{k}", bufs=2)
                           for k in range(KD)]
                    for k in range(KD):
                        nc.scalar.dma_start(
                            w1t[k][:],
                            t["w1_d"][k * P:(k + 1) * P,
                                      blk * MFB * P:(blk + 1) * MFB * P])
                    w1s = [w1t[k][:] for k in range(KD)]
                for j in range(MFB):
                    mf = blk * MFB + j
                    ps = pps.tile([P, TS], F32, tag="ps", name="g")
                    for k in range(KD):
                        nc.tensor.matmul(
                            ps[:, c0t:c1t], w1s[k][:, j * P:(j + 1) * P],
                            h2T[k][:, c0t:c1t],
                            start=(k == 0), stop=(k == KD - 1))
                    nc.scalar.activation(gT[mf][:, c0t:c1t],
                                         ps[:, c0t:c1t], AF.Gelu,
                                         bias=b1p[:, mf:mf + 1])
        w1p_cm.__exit__(None, None, None)
        h2tp_cm.__exit__(None, None, None)

        # second FFN matmul in two mt-halves (PSUM budget), W2 streamed twice
        with tc.tile_pool(name="w2st", bufs=4) as w2p, \
             tc.tile_pool(name="fPS", bufs=1, space="PSUM") as fps, \
             tc.tile_pool(name="ySB", bufs=2) as ysb:
            for half in range(2):
                f_ps = [fps.tile([P, D], F32, tag=f"f{i}", name=f"f{half}_{i}")
                        for i in range(2)]
                for i in range(2):
                    for n in range(2):
                        nc.tensor.matmul(
                            f_ps[i][:, n * TS:(n + 1) * TS],
                            ones128[:], b2row[:, n * TS:(n + 1) * TS],
                            start=True, stop=False)
                for k2 in range(KF):
                    w2t = w2p.tile([P, D], BF16, tag="w2", name="w2")
                    nc.scalar.dma_start(
                        w2t[:], t["w2_d"][k2 * P:(k2 + 1) * P, :])
                    for i in range(2):
                        mt = 2 * half + i
                        for n in range(2):
                            nc.tensor.matmul(
                                f_ps[i][:, n * TS:(n + 1) * TS],
                                gT[k2][:, mt * P:(mt + 1) * P],
                                w2t[:, n * TS:(n + 1) * TS],
                                start=False, stop=(k2 == KF - 1))
                for i in range(2):
                    mt = 2 * half + i
                    yt = ysb.tile([P, D], F32, tag="y", name="y")
                    nc.vector.tensor_tensor(out=yt[:], in0=f_ps[i][:],
                                            in1=x2[mt][:], op=ALU.add)
                    nc.gpsimd.dma_start(t["y_d"][mt * P:(mt + 1) * P, :],
                                        yt[:])
        gtp_cm.__exit__(None, None, None)
        x2p_cm.__exit__(None, None, None)
        pps_cm.__exit__(None, None, None)
        pfw1_cm.__exit__(None, None, None)
        rsb_cm.__exit__(None, None, None)
        sc_cm.__exit__(None, None, None)


def _own_idx(r):
    return np.concatenate([
        np.arange(qc * TS + r * P, qc * TS + r * P + P) for qc in range(G)
    ])


def _in_maps(inputs):
    f32 = np.float32
    bf16 = ml_dtypes.bfloat16
    x = np.asarray(inputs["x"], f32)
    g1 = np.asarray(inputs["ln1_g"], f32)
    be1 = np.asarray(inputs["ln1_b"], f32)
    g2 = np.asarray(inputs["ln2_g"], f32)
    be2 = np.asarray(inputs["ln2_b"], f32)
    # fold LN gains/biases into the following projections (exact rewrite)
    wq = g1[:, None] * np.asarray(inputs["Wq"], f32)
    wk = g1[:, None] * np.asarray(inputs["Wk"], f32)
    wv = g1[:, None] * np.asarray(inputs["Wv"], f32)
    bqf = np.asarray(inputs["bq"], f32) + be1 @ np.asarray(inputs["Wq"], f32)
    bkf = np.asarray(inputs["bk"], f32) + be1 @ np.asarray(inputs["Wk"], f32)
    bvf = np.asarray(inputs["bv"], f32) + be1 @ np.asarray(inputs["Wv"], f32)
    wo = np.ascontiguousarray(np.asarray(inputs["Wo"], f32))
    w1f = np.asarray(inputs["W1"], f32)
    b1f = np.asarray(inputs["b1"], f32) + be2 @ w1f
    w1 = np.ascontiguousarray(g2[:, None] * w1f).astype(bf16)
    w2 = np.ascontiguousarray(np.asarray(inputs["W2"], f32)).astype(bf16)
    b2 = np.ascontiguousarray(np.asarray(inputs["b2"], f32)).astype(bf16)
    bo = np.asarray(inputs["bo"], f32)
    maps = []
    for c in range(NC):
        b, r = c // G, c % G
        c0 = r * CC
        idx = _own_idx(r)
        m = {
            "x": np.ascontiguousarray(x[b]),
            "xr": np.ascontiguousarray(x[b][idx] + bo[None, :]),
            "Wq": np.ascontiguousarray(wq[:, c0:c0 + CC]).astype(bf16),
            "Wk": np.ascontiguousarray(wk[:, c0:c0 + CC]).astype(bf16),
            "Wv": np.ascontiguousarray(wv[:, c0:c0 + CC]).astype(bf16),
            "bq": np.ascontiguousarray(bqf[c0:c0 + CC]),
            "bk": np.ascontiguousarray(bkf[c0:c0 + CC]),
            "bv": np.ascontiguousarray(bvf[c0:c0 + CC]),
            "Wo": np.ascontiguousarray(wo[c0:c0 + CC, :]).astype(bf16),
            "W1": w1,
            "b1": np.ascontiguousarray(b1f),
            "W2": w2,
            "b2": b2,
        }
        maps.append(m)
    return maps


def _run(inputs, trace=False):
    if "nc" not in _CACHE:
        _CACHE["nc"] = build()
    nc = _CACHE["nc"]
    maps = _in_maps(inputs)
    res = run_bass_kernel_spmd(nc, maps, list(range(NC)), trace=trace)
    out = np.empty((B, S, D), np.float32)
    for c in range(NC):
        b, r = c // G, c % G
        out[b, _own_idx(r), :] = res.results[c]["y"]
    return out, res


def kernel(**inputs):
    out, _ = _run(inputs, trace=False)
    return out


if __name__ == "__main__":
    build()
    print("build OK")


# revision 21
# speedup vs baseline: 1.0725x; 1.0725x over previous
"""Trainium2 Bass kernel for a dense transformer block (B=2,S=2048,D=1024,H=16,DFF=4096).

Sharding across 8 NeuronCores (no AllGathers):
  core c: batch b=c//4, group rank r=c%4, replica groups [[0,1,2,3],[4,5,6,7]].
  - x replicated within the group; LN1 + transpose computed redundantly over
    the full sequence (cheap) so QKV needs no collective.
  - Attention: head-parallel (4 heads/core, full causal sequence), fused in a
    per-query-strip pipeline with LN1/QKV.
  - out_proj: each core computes the partial y contribution of its heads for
    the strip, then a small ReduceScatter(add) both sums the partials and
    scatters tokens -- 4 chunked RS ops overlap with attention of later strips.
  - residual + LN2 + FFN: token-parallel on the core's 512 owned (interleaved)
    tokens with full FFN weights (no collective).
Matmul operands are bf16 (weights converted host-side); accumulation and the
residual spine stay fp32.
"""
import sys

sys.path.insert(0, "/opt/trn_rl_repo")

import numpy as np
import ml_dtypes

import concourse.bass as bass
import concourse.mybir as mybir
import concourse.tile as tile
from concourse import bacc
from concourse.bass_utils import run_bass_kernel_spmd
from concourse.masks import make_identity

AF = mybir.ActivationFunctionType
ALU = mybir.AluOpType
F32 = mybir.dt.float32
F32R = mybir.dt.float32r
BF16 = mybir.dt.bfloat16

B, S, D, H = 2, 2048, 1024, 16
DH = D // H          # 64
DFF = 4 * D          # 4096
EPS = 1e-5
NC = 8               # cores
G = 4                # cores per group (per batch)
TS = S // G          # 512 tokens per query strip
HC = H // G          # 4 heads per core
CC = HC * DH         # 256 head-columns per core
P = 128
KD = D // P          # 8 k-tiles over D
KF = DFF // P        # 32 k-tiles over DFF
NT = TS // P         # 4 token tiles per strip
GROUPS = [[0, 1, 2, 3], [4, 5, 6, 7]]
MFB = 4              # mf tiles per W1 stream block

_CACHE = {}


def build():
    nc = bacc.Bacc(None)

    io = {}
    dp = nc.declare_dram_parameter
    io["x_d"] = dp("x", [S, D], F32, isOutput=False)
    io["xr_d"] = dp("xr", [TS, D], F32, isOutput=False)
    io["wq_d"] = dp("Wq", [D, CC], BF16, isOutput=False)
    io["wk_d"] = dp("Wk", [D, CC], BF16, isOutput=False)
    io["wv_d"] = dp("Wv", [D, CC], BF16, isOutput=False)
    io["bq_d"] = dp("bq", [CC], F32, isOutput=False)
    io["bk_d"] = dp("bk", [CC], F32, isOutput=False)
    io["bv_d"] = dp("bv", [CC], F32, isOutput=False)
    io["wo_d"] = dp("Wo", [CC, D], BF16, isOutput=False)
    io["w1_d"] = dp("W1", [D, DFF], BF16, isOutput=False)
    io["b1_d"] = dp("b1", [DFF], F32, isOutput=False)
    io["w2_d"] = dp("W2", [DFF, D], BF16, isOutput=False)
    io["b2_d"] = dp("b2", [D], BF16, isOutput=False)
    io["y_d"] = dp("y", [TS, D], F32, isOutput=True)

    for qc in range(G):
        io[f"rs_in{qc}"] = nc.dram_tensor(f"rs_in{qc}", [TS, D], BF16)
        io[f"rs_out{qc}"] = nc.dram_tensor(f"rs_out{qc}", [P, D], BF16)

    with tile.TileContext(nc) as tc:
        _body(nc, tc, io)
    nc.compile()
    return nc


def _body(nc, tc, t):
    with tc.tile_pool(name="const", bufs=1) as cst:
        # ---------------- constants ----------------
        ident = cst.tile([P, P], F32)
        make_identity(nc, ident[:])

        ones128 = cst.tile([1, P], BF16)        # K=1 lhsT (M=128 tokens)
        nc.gpsimd.memset(ones128[:], 1.0)
        onescol4 = cst.tile([P, HC, 1], BF16)
        nc.gpsimd.memset(onescol4[:], 1.0)
        epsc = cst.tile([P, 1], F32)
        nc.gpsimd.memset(epsc[:], EPS)

        # doubled causal masks (one per diagonal shift), mask||mask layout so a
        # single DVE op masks a two-head [128, 1024] pair tile.
        maskd = {}
        for sh in (0, -128, -256, -384):
            md = cst.tile([P, 2 * TS], BF16, tag=f"maskd{sh}", name=f"maskd{sh}")
            nc.gpsimd.memset(md[:], 1.0)
            for half in range(2):
                nc.gpsimd.affine_select(
                    out=md[:, half * TS:(half + 1) * TS],
                    in_=md[:, half * TS:(half + 1) * TS],
                    compare_op=ALU.is_ge, fill=0.0, base=sh,
                    pattern=[[1, TS]], channel_multiplier=-1,
                )
            maskd[sh] = md

        # bq/bk as per-partition [128, 2] (column-tile-major) for psum eviction
        bqp = cst.tile([P, 2], F32)
        bkp = cst.tile([P, 2], F32)
        nc.gpsimd.dma_start(bqp[:], t["bq_d"].rearrange("(m p) -> p m", p=P))
        nc.gpsimd.dma_start(bkp[:], t["bk_d"].rearrange("(m p) -> p m", p=P))
        # bv broadcast across partitions for the v eviction add
        bvrow = cst.tile([1, CC], F32)
        nc.gpsimd.dma_start(bvrow[:], t["bv_d"][None, :])
        bvb = cst.tile([P, CC], F32)
        nc.gpsimd.partition_broadcast(bvb[:], bvrow[:])
        # b1 as per-partition [128, KF] for the gelu bias operand
        b1p = cst.tile([P, KF], F32)
        nc.gpsimd.dma_start(b1p[:], t["b1_d"].rearrange("(k p) -> p k", p=P))
        b2row = cst.tile([1, D], BF16)
        nc.gpsimd.dma_start(b2row[:], t["b2_d"][None, :])
        # residual strip (x at owned tokens, bo folded in host-side)
        xr = [cst.tile([P, D], F32, tag=f"xr{i}", name=f"xr{i}")
              for i in range(NT)]
        for i in range(NT):
            nc.gpsimd.dma_start(xr[i][:], t["xr_d"][i * P:(i + 1) * P, :])

        # ---------------- helpers ----------------
        def layernorm_tile(xt, dst, sc):
            # var = E[x^2] - mu^2 (safe: |mu| << std for this data); stats
            # chain kept on DVE to minimize cross-engine semaphore hops:
            # act does the big Square+accum pass and the Rsqrt table lookup.
            mu = sc.tile([P, 1], F32, tag="mu", name="mu")
            nc.vector.tensor_reduce(out=mu[:], in_=xt[:], op=ALU.add,
                                    axis=mybir.AxisListType.X)
            sq = sc.tile([P, D], F32, tag="sq", name="sq")
            sumsq = sc.tile([P, 1], F32, tag="sumsq", name="sumsq")
            nc.scalar.activation(sq[:], xt[:], AF.Square, accum_out=sumsq[:])
            mus = sc.tile([P, 1], F32, tag="mus", name="mus")
            nc.vector.tensor_scalar(out=mus[:], in0=mu[:], scalar1=1.0 / D,
                                    scalar2=None, op0=ALU.mult)
            mu2 = sc.tile([P, 1], F32, tag="mu2", name="mu2")
            nc.vector.tensor_tensor(out=mu2[:], in0=mus[:], in1=mus[:],
                                    op=ALU.mult)
            vpe = sc.tile([P, 1], F32, tag="vpe", name="vpe")
            nc.vector.tensor_scalar(out=vpe[:], in0=sumsq[:],
                                    scalar1=1.0 / D, scalar2=mu2[:],
                                    op0=ALU.mult, op1=ALU.subtract)
            std = sc.tile([P, 1], F32, tag="std", name="std")
            nc.scalar.activation(std[:], vpe[:], AF.Sqrt, bias=epsc[:])
            inv = sc.tile([P, 1], F32, tag="inv", name="inv")
            nc.vector.reciprocal(inv[:], std[:])
            nc.vector.tensor_scalar(out=dst[:], in0=xt[:],
                                    scalar1=mus[:], scalar2=inv[:],
                                    op0=ALU.subtract, op1=ALU.mult)

        # ============ phase 1: fused LN1/QKV/attention/out_proj + RS ========
        # pool entry order is chosen so exits can be LIFO
        sc_cm = tc.tile_pool(name="lnS", bufs=2)
        sc = sc_cm.__enter__()
        rsb_cm = tc.tile_pool(name="rsSB", bufs=4)
        rsb = rsb_cm.__enter__()
        pfw1_cm = tc.tile_pool(name="pfW1", bufs=1)
        pfw1 = pfw1_cm.__enter__()
        pps_cm = tc.tile_pool(name="psA", bufs=2, space="PSUM")
        pps = pps_cm.__enter__()
        xsp_cm = tc.tile_pool(name="xsP", bufs=5)
        xsp = xsp_cm.__enter__()
        hTp_cm = tc.tile_pool(name="hTP", bufs=1)
        hTp = hTp_cm.__enter__()
        qkv_cm = tc.tile_pool(name="qkvP", bufs=1)
        qkv = qkv_cm.__enter__()
        wp_cm = tc.tile_pool(name="wqkv", bufs=1)
        wp = wp_cm.__enter__()
        asb_cm = tc.tile_pool(name="attnSB", bufs=2)
        asb = asb_cm.__enter__()
        scp_cm = tc.tile_pool(name="scPS", bufs=2, space="PSUM")
        scp = scp_cm.__enter__()
        avp_cm = tc.tile_pool(name="avPS", bufs=1, space="PSUM")
        avp = avp_cm.__enter__()

        hT = hTp.tile([P, KD, S], BF16, tag="hT", name="hT")
        qT = [qkv.tile([P, S], BF16, tag=f"qT{m}", name=f"qT{m}") for m in range(2)]
        kT = [qkv.tile([P, S], BF16, tag=f"kT{m}", name=f"kT{m}") for m in range(2)]
        vo = [qkv.tile([P, HC, DH + 1], BF16, tag=f"vo{tm}", name=f"vo{tm}")
              for tm in range(S // P)]
        att = [qkv.tile([P, S], BF16, tag=f"att{hp}", name=f"att{hp}")
               for hp in range(2)]

        # first strip's x tiles; later strips prefetched inside the loop
        xs = {}
        for mt in range(NT):
            xs[mt] = xsp.tile([P, D], F32, tag="xs", name=f"xs{mt}")
            nc.sync.dma_start(xs[mt][:], t["x_d"][mt * P:(mt + 1) * P, :])

        # qkv weights + wo (sync queue, behind first x tiles)
        wq = [wp.tile([P, CC], BF16, tag=f"wq{k}", name=f"wq{k}") for k in range(KD)]
        wk = [wp.tile([P, CC], BF16, tag=f"wk{k}", name=f"wk{k}") for k in range(KD)]
        wv = [wp.tile([P, CC], BF16, tag=f"wv{k}", name=f"wv{k}") for k in range(KD)]
        for k in range(KD):
            nc.sync.dma_start(wq[k][:], t["wq_d"][k * P:(k + 1) * P, :])
            nc.sync.dma_start(wk[k][:], t["wk_d"][k * P:(k + 1) * P, :])
            nc.sync.dma_start(wv[k][:], t["wv_d"][k * P:(k + 1) * P, :])
        wo = [wp.tile([P, D], BF16, tag=f"wo{hp}", name=f"wo{hp}") for hp in range(2)]
        for hp in range(2):
            nc.sync.dma_start(wo[hp][:], t["wo_d"][hp * P:(hp + 1) * P, :])

        # FFN W1 prefetch: first block on the scalar queue
        w1s0 = [pfw1.tile([P, MFB * P], BF16, tag=f"w1s{k}", name=f"w1s{k}")
                for k in range(KD)]
        for k in range(KD):
            nc.scalar.dma_start(w1s0[k][:], t["w1_d"][k * P:(k + 1) * P,
                                                      0:MFB * P])

        # ---- LN1 + transpose + QKV, interleaved per strip ----
        for qc in range(G):
            if qc + 1 < G:
                for mt in range(NT):
                    mt16 = (qc + 1) * NT + mt
                    xs[mt16] = xsp.tile([P, D], F32, tag="xs", name=f"xs{mt16}")
                    nc.sync.dma_start(xs[mt16][:],
                                      t["x_d"][mt16 * P:(mt16 + 1) * P, :])
            for mt in range(NT):
                mt16 = qc * NT + mt
                h = sc.tile([P, D], F32, tag="h", name=f"h{mt16}", bufs=4)
                layernorm_tile(xs[mt16], h, sc)
                for g2 in range(2):        # k groups 0-3 / 4-7
                    ps = pps.tile([P, TS], F32, tag="ps", name="tps")
                    for kk in range(4):
                        k = 4 * g2 + kk
                        nc.tensor.transpose(ps[:, kk * P:(kk + 1) * P],
                                            h[:, k * P:(k + 1) * P],
                                            ident[:])
                    nc.vector.tensor_copy(
                        hT[:, 4 * g2:4 * g2 + 4,
                           mt16 * P:(mt16 + 1) * P],
                        ps[:].rearrange("p (k c) -> p k c", k=4))
            for (w_sb, b_sb, out_sb) in ((wq, bqp, qT), (wk, bkp, kT)):
                for m in range(2):
                    ps = pps.tile([P, TS], F32, tag="ps", name="pps")
                    for k in range(KD):
                        nc.tensor.matmul(
                            ps[:], w_sb[k][:, m * P:(m + 1) * P],
                            hT[:, k, qc * TS:(qc + 1) * TS],
                            start=(k == 0), stop=(k == KD - 1))
                    nc.vector.tensor_scalar(
                        out=out_sb[m][:, qc * TS:(qc + 1) * TS],
                        in0=ps[:], scalar1=b_sb[:, m:m + 1],
                        scalar2=None, op0=ALU.add)
            for mt in range(NT):
                tm = qc * NT + mt
                ps = pps.tile([P, TS], F32, tag="ps", name="vps")
                for k in range(KD):
                    nc.tensor.matmul(
                        ps[:, 0:CC], hT[:, k, tm * P:(tm + 1) * P],
                        wv[k][:], start=(k == 0), stop=(k == KD - 1))
                nc.vector.tensor_tensor(
                    out=vo[tm][:, :, 0:DH],
                    in0=ps[:, 0:CC].rearrange("p (h e) -> p h e", h=HC),
                    in1=bvb[:].rearrange("p (h e) -> p h e", h=HC),
                    op=ALU.add)
                nc.gpsimd.tensor_copy(vo[tm][:, :, DH:DH + 1], onescol4[:])

        for qc in range(G):
            # ---- attention for this strip (all 4 heads) ----
            kt_max = 4 * qc + 3
            for hp in range(HC // 2):      # head pairs at PE rows 0/64
                avs = avp.tile([DH + 1, 2, TS], F32, tag="av", name=f"av{hp}")
                for kt in range(kt_max + 1):
                    # diag blocks: only columns >= v0 are ever read
                    w0 = P * max(0, kt - 4 * qc)
                    sc_ps = scp.tile([P, 2, TS], F32, tag="scp", name="scp")
                    for j in range(2):
                        h_i = 2 * hp + j
                        m = h_i // 2
                        o = (h_i % 2) * DH
                        nc.tensor.matmul(
                            sc_ps[:, j, w0:],
                            kT[m][o:o + DH, kt * P:(kt + 1) * P],
                            qT[m][o:o + DH, qc * TS + w0:(qc + 1) * TS],
                            start=True, stop=True)
                    e_r = asb.tile([P, 2, TS], BF16, tag="erp", name="erp")
                    if kt < 4 * qc:
                        v0 = 0      # valid columns start
                        nc.scalar.activation(
                            e_r[:].rearrange("p a b -> p (a b)"),
                            sc_ps[:].rearrange("p a b -> p (a b)"),
                            AF.Exp, scale=0.125)
                    else:
                        # diag block, shift s=-128*d: cols < 128*d
                        # are fully masked -- never compute/read them
                        d = kt - 4 * qc
                        v0 = P * d
                        nc.scalar.activation(
                            e_r[:, :, v0:], sc_ps[:, :, v0:],
                            AF.Exp, scale=0.125)
                        sh = 512 * qc - 128 * kt
                        mdv = maskd[sh][:].rearrange("p (a b) -> p a b", a=2)
                        nc.vector.tensor_tensor(
                            out=e_r[:, :, v0:], in0=e_r[:, :, v0:],
                            in1=mdv[:, :, v0:], op=ALU.mult)
                    for j in range(2):
                        h_i = 2 * hp + j
                        nc.tensor.matmul(avs[:, j, v0:],
                                         vo[kt][:, h_i, :],
                                         e_r[:, j, v0:],
                                         start=(kt == 0),
                                         stop=(kt == kt_max))
                # normalize + write into persistent att tiles (bf16)
                un = asb.tile([DH + 1, 2, TS], F32, tag="un", name="un",
                              bufs=1)
                nc.vector.tensor_copy(un[:], avs[:])
                rec = asb.tile([1, 2, TS], F32, tag="rec", name="rec", bufs=1)
                nc.vector.reciprocal(rec[:].rearrange("p a b -> p (a b)"),
                                     un[DH:DH + 1, :, :].rearrange(
                                         "p a b -> p (a b)"))
                rb = asb.tile([DH, 2, TS], F32, tag="rb", name="rb", bufs=1)
                nc.gpsimd.partition_broadcast(
                    rb[:].rearrange("p a b -> p (a b)"),
                    rec[:].rearrange("p a b -> p (a b)"))
                for j in range(2):
                    nc.vector.tensor_tensor(
                        out=att[hp][j * DH:(j + 1) * DH,
                                    qc * TS:(qc + 1) * TS],
                        in0=un[0:DH, j, :], in1=rb[:, j, :], op=ALU.mult)

            # ---- out_proj partial for this strip + ReduceScatter ----
            for mt in range(NT):
                ri = rsb.tile([P, D], BF16, tag="ri", name=f"ri{qc}_{mt}")
                for n in range(2):
                    ps = pps.tile([P, TS], F32, tag="ps", name="ops")
                    for hp in range(2):
                        nc.tensor.matmul(
                            ps[:],
                            att[hp][:, qc * TS + mt * P:qc * TS + (mt + 1) * P],
                            wo[hp][:, n * TS:(n + 1) * TS],
                            start=(hp == 0), stop=(hp == 1))
                    nc.scalar.mul(ri[:, n * TS:(n + 1) * TS], ps[:], 1.0)
                nc.sync.dma_start(t[f"rs_in{qc}"][mt * P:(mt + 1) * P, :], ri[:])
            nc.gpsimd.collective_compute(
                "ReduceScatter", ALU.add, ins=[t[f"rs_in{qc}"][:]],
                outs=[t[f"rs_out{qc}"][:]], replica_groups=GROUPS,
            )

        # attention-phase pools done (LIFO)
        avp_cm.__exit__(None, None, None)
        scp_cm.__exit__(None, None, None)
        asb_cm.__exit__(None, None, None)
        wp_cm.__exit__(None, None, None)
        qkv_cm.__exit__(None, None, None)
        hTp_cm.__exit__(None, None, None)
        xsp_cm.__exit__(None, None, None)

        # ============ phase 2: residual + LN2 + FFN on owned tokens ========
        x2p_cm = tc.tile_pool(name="x2P", bufs=1)
        x2p = x2p_cm.__enter__()
        gtp_cm = tc.tile_pool(name="gTP", bufs=1)
        gtp = gtp_cm.__enter__()
        h2tp_cm = tc.tile_pool(name="h2TP", bufs=1)
        h2tp = h2tp_cm.__enter__()
        x2 = [x2p.tile([P, D], F32, tag=f"x2{i}", name=f"x2{i}")
              for i in range(NT)]
        h2T = [h2tp.tile([P, TS], BF16, tag=f"h2T{k}", name=f"h2T{k}")
               for k in range(KD)]
        h2s = []
        for qc in range(G):
            ro = rsb.tile([P, D], BF16, tag="ro", name=f"ro{qc}", bufs=1)
            nc.gpsimd.dma_start(ro[:], t[f"rs_out{qc}"][:])
            nc.vector.tensor_tensor(out=x2[qc][:], in0=ro[:], in1=xr[qc][:],
                                    op=ALU.add)
            h2 = sc.tile([P, D], F32, tag="h2", name=f"h2{qc}", bufs=4)
            layernorm_tile(x2[qc], h2, sc)
            h2s.append(h2)
            if qc == 2:
                for k in range(KD):
                    ps = pps.tile([P, TS], F32, tag="ps", name="tps2")
                    for q3 in range(3):
                        nc.tensor.transpose(ps[:, q3 * P:(q3 + 1) * P],
                                            h2s[q3][:, k * P:(k + 1) * P],
                                            ident[:])
                    nc.vector.tensor_copy(h2T[k][:, 0:3 * P],
                                          ps[:, 0:3 * P])
        for k in range(KD):
            ps = pps.tile([P, TS], F32, tag="ps", name="tps3")
            nc.tensor.transpose(ps[:, 0:P],
                                h2s[3][:, k * P:(k + 1) * P], ident[:])
            nc.vector.tensor_copy(h2T[k][:, 3 * P:4 * P], ps[:, 0:P])
        # ---- FFN ----
        gT = [gtp.tile([P, TS], BF16, tag=f"gT{mf}", name=f"gT{mf}")
              for mf in range(KF)]
        w1p_cm = tc.tile_pool(name="w1st", bufs=1)
        w1p = w1p_cm.__enter__()
        for blk in range(KF // MFB):
            if blk < 1:
                w1s = [w1s0[k][:, 0:MFB * P] for k in range(KD)]
            else:
                w1t = [w1p.tile([P, MFB * P], BF16,
                                tag=f"w1b{k}", name=f"w1b{k}", bufs=2)
                       for k in range(KD)]
                for k in range(KD):
                    nc.scalar.dma_start(
                        w1t[k][:],
                        t["w1_d"][k * P:(k + 1) * P,
                                  blk * MFB * P:(blk + 1) * MFB * P])
                w1s = [w1t[k][:] for k in range(KD)]
            for j in range(MFB):
                mf = blk * MFB + j
                ps = pps.tile([P, TS], F32, tag="ps", name="g")
                for k in range(KD):
                    nc.tensor.matmul(
                        ps[:], w1s[k][:, j * P:(j + 1) * P],
                        h2T[k][:], start=(k == 0), stop=(k == KD - 1))
                nc.scalar.activation(gT[mf][:], ps[:], AF.Gelu,
                                     bias=b1p[:, mf:mf + 1])
        w1p_cm.__exit__(None, None, None)
        h2tp_cm.__exit__(None, None, None)

        # second FFN matmul in two mt-halves (PSUM budget), W2 streamed twice
        with tc.tile_pool(name="w2st", bufs=4) as w2p, \
             tc.tile_pool(name="fPS", bufs=1, space="PSUM") as fps, \
             tc.tile_pool(name="ySB", bufs=2) as ysb:
            for half in range(2):
                f_ps = [fps.tile([P, D], F32, tag=f"f{i}", name=f"f{half}_{i}")
                        for i in range(2)]
                for i in range(2):
                    for n in range(2):
                        nc.tensor.matmul(
                            f_ps[i][:, n * TS:(n + 1) * TS],
                            ones128[:], b2row[:, n * TS:(n + 1) * TS],
                            start=True, stop=False)
                for k2 in range(KF):
                    w2t = w2p.tile([P, D], BF16, tag="w2", name="w2")
                    nc.scalar.dma_start(
                        w2t[:], t["w2_d"][k2 * P:(k2 + 1) * P, :])
                    for i in range(2):
                        mt = 2 * half + i
                        for n in range(2):
                            nc.tensor.matmul(
                                f_ps[i][:, n * TS:(n + 1) * TS],
                                gT[k2][:, mt * P:(mt + 1) * P],
                                w2t[:, n * TS:(n + 1) * TS],
                                start=False, stop=(k2 == KF - 1))
                for i in range(2):
                    mt = 2 * half + i
                    yt = ysb.tile([P, D], F32, tag="y", name="y")
                    nc.vector.tensor_tensor(out=yt[:], in0=f_ps[i][:],
                                            in1=x2[mt][:], op=ALU.add)
                    nc.gpsimd.dma_start(t["y_d"][mt * P:(mt + 1) * P, :],
                                        yt[:])
        gtp_cm.__exit__(None, None, None)
        x2p_cm.__exit__(None, None, None)
        pps_cm.__exit__(None, None, None)
        pfw1_cm.__exit__(None, None, None)
        rsb_cm.__exit__(None, None, None)
        sc_cm.__exit__(None, None, None)


def _own_idx(r):
    return np.concatenate([
        np.arange(qc * TS + r * P, qc * TS + r * P + P) for qc in range(G)
    ])


def _in_maps(inputs):
    f32 = np.float32
    bf16 = ml_dtypes.bfloat16
    x = np.asarray(inputs["x"], f32)
    g1 = np.asarray(inputs["ln1_g"], f32)
    be1 = np.asarray(inputs["ln1_b"], f32)
    g2 = np.asarray(inputs["ln2_g"], f32)
    be2 = np.asarray(inputs["ln2_b"], f32)
    # fold LN gains/biases into the following projections (exact rewrite)
    wq = g1[:, None] * np.asarray(inputs["Wq"], f32)
    wk = g1[:, None] * np.asarray(inputs["Wk"], f32)
    wv = g1[:, None] * np.asarray(inputs["Wv"], f32)
    bqf = np.asarray(inputs["bq"], f32) + be1 @ np.asarray(inputs["Wq"], f32)
    bkf = np.asarray(inputs["bk"], f32) + be1 @ np.asarray(inputs["Wk"], f32)
    bvf = np.asarray(inputs["bv"], f32) + be1 @ np.asarray(inputs["Wv"], f32)
    wo = np.ascontiguousarray(np.asarray(inputs["Wo"], f32))
    w1f = np.asarray(inputs["W1"], f32)
    b1f = np.asarray(inputs["b1"], f32) + be2 @ w1f
    w1 = np.ascontiguousarray(g2[:, None] * w1f).astype(bf16)
    w2 = np.ascontiguousarray(np.asarray(inputs["W2"], f32)).astype(bf16)
    b2 = np.ascontiguousarray(np.asarray(inputs["b2"], f32)).astype(bf16)
    bo = np.asarray(inputs["bo"], f32)
    maps = []
    for c in range(NC):
        b, r = c // G, c % G
        c0 = r * CC
        idx = _own_idx(r)
        m = {
            "x": np.ascontiguousarray(x[b]),
            "xr": np.ascontiguousarray(x[b][idx] + bo[None, :]),
            "Wq": np.ascontiguousarray(wq[:, c0:c0 + CC]).astype(bf16),
            "Wk": np.ascontiguousarray(wk[:, c0:c0 + CC]).astype(bf16),
            "Wv": np.ascontiguousarray(wv[:, c0:c0 + CC]).astype(bf16),
            "bq": np.ascontiguousarray(bqf[c0:c0 + CC]),
            "bk": np.ascontiguousarray(bkf[c0:c0 + CC]),
            "bv": np.ascontiguousarray(bvf[c0:c0 + CC]),
            "Wo": np.ascontiguousarray(wo[c0:c0 + CC, :]).astype(bf16),
            "W1": w1,
            "b1": np.ascontiguousarray(b1f),
            "W2": w2,
            "b2": b2,
        }
        maps.append(m)
    return maps


def _run(inputs, trace=False):
    if "nc" not in _CACHE:
        _CACHE["nc"] = build()
    nc = _CACHE["nc"]
    maps = _in_maps(inputs)
    res = run_bass_kernel_spmd(nc, maps, list(range(NC)), trace=trace)
    out = np.empty((B, S, D), np.float32)
    for c in range(NC):
        b, r = c // G, c % G
        out[b, _own_idx(r), :] = res.results[c]["y"]
    return out, res


def kernel(**inputs):
    out, _ = _run(inputs, trace=False)
    return out


if __name__ == "__main__":
    build()
    print("build OK")
